# revision 6
# baseline (speedup 1.0000x reference)
"""Multi-head attention (COAMultiHeadAttention) on 8 Trainium2 NeuronCores.

Sharding: batch x head-group. Core c (0..7) handles batch b = c//4 and head
group g = c%4 (4 of 16 heads, i.e. a 256-wide slice of the 1024-dim model).

Fully-fused single-pipeline device program. The ScalarE exp stream
(64 iters x 2 heads x [128,1024] ACTIVATE = ~147us/core) is the critical
path; everything else (DMA, Q/K/V projections, PV, output projection) is
scheduled into the PE/DVE slack underneath it:
  - x tensors are DMA'd in 512-token groups and Q^T/K^T projected
    incrementally, so the first QK->exp starts after ~1/4 of the input
    has landed instead of after full phase-A.
  - Each attention block (head-pair x q-half) STAGES its exp outputs
    (pt tiles, SBUF) for the first STAGE iters without running PV. This
    leaves the 4 "win" PSUM banks free: projection / output-projection
    jobs borrow them as accumulators between QKs. From iter STAGE on,
    PV runs at 2 chunks/iter to catch up, with the final chunk + the
    normalization tail software-pipelined into the next block.
  - Scores are score-transposed (k-part, q-free); PV carries an extra
    ones column in V that yields the softmax denominators for free.
    Head pairs ride different PE row groups (concurrent QK matmuls).
  - Normalization uses a DRAM-bounce partition broadcast of reciprocal
    denominators; output projection partials att_n @ wo[:, slice]^T go
    out per 128-token chunk as soon as a q-half is normalized.
Host sums the 4 partials per batch in fp32 and adds bo.

Everything the device sees is pre-laid-out by the host (transposes, head
slicing, bias replication), so the device program is identical on all cores.
"""

import os

import ml_dtypes
import numpy as np

import concourse.bass as bass  # noqa: F401  (AP types resolve through this import)
import concourse.mybir as mybir
import concourse.tile as tile
from concourse import bacc, bass_utils

F32 = mybir.dt.float32
F32R = mybir.dt.float32r
BF16 = mybir.dt.bfloat16
AT = mybir.ActivationFunctionType
ALU = mybir.AluOpType

B = 2
T = 2048
D = 1024
N_HEADS = 16
HEAD_DIM = 64
N_CORES = 8
S = 256            # per-core slice of the model dim (4 heads)
NHL = 4            # heads per core
P = 128
DC = D // P        # 8 contraction chunks for the projections
TC = T // P        # 16 token chunks
SCALE = 1.0 / np.sqrt(HEAD_DIM)

_CACHE = {}
LAST_STATS = {}


def _patch_walrus_flags():
    """Enable walrus's LDWEIGHTS optimization (dedupe/pipeline weight loads).
    bass_utils hardcodes --enable-ldw-opt=false; without the opt every MATMUL
    serially waits ~140ns for its own LDWEIGHTS, which costs ~100us here."""
    if getattr(bass_utils, "_ldw_patched", False):
        return
    orig = bass_utils.run_command

    def patched(argv, **kw):
        argv = ["--enable-ldw-opt=true" if a == "--enable-ldw-opt=false" else a
                for a in argv]
        return orig(argv, **kw)

    bass_utils.run_command = patched
    bass_utils._ldw_patched = True


def _build_program():
    nc = bacc.Bacc("TRN2", target_bir_lowering=False, debug=False)

    xq_d = nc.dram_tensor("xq", [P, DC, T], BF16, kind="ExternalInput").ap()
    xk_d = nc.dram_tensor("xk", [P, DC, T], BF16, kind="ExternalInput").ap()
    xv_d = nc.dram_tensor("xv", [P, DC, T], BF16, kind="ExternalInput").ap()
    wqt_d = nc.dram_tensor("wqt", [P, DC, S], BF16, kind="ExternalInput").ap()
    wkt_d = nc.dram_tensor("wkt", [P, DC, S], BF16, kind="ExternalInput").ap()
    wvt_d = nc.dram_tensor("wvt", [P, DC, S], BF16, kind="ExternalInput").ap()
    bq_d = nc.dram_tensor("bq", [P, 2], F32, kind="ExternalInput").ap()
    bk_d = nc.dram_tensor("bk", [P, 2], F32, kind="ExternalInput").ap()
    bv_d = nc.dram_tensor("bv", [P, NHL, HEAD_DIM], F32, kind="ExternalInput").ap()
    wot_d = nc.dram_tensor("wot", [P, 2, D], BF16, kind="ExternalInput").ap()
    out_d = nc.dram_tensor("out_part", [TC, P, D], BF16, kind="ExternalOutput").ap()
    sums_d = nc.dram_tensor("sums_scr", [NHL, T], F32).ap()
    rsums_d = nc.dram_tensor("rsums_scr", [NHL, T], F32).ap()

    with tile.TileContext(nc) as tc:
        _body(tc, xq_d, xk_d, xv_d, wqt_d, wkt_d, wvt_d,
              bq_d, bk_d, bv_d, wot_d, out_d, sums_d, rsums_d)
    nc.compile()
    return nc


QH = 1024          # q-width of one attention block (exp free-dim)
TG = 512           # token-group granularity for x DMA / projections
STAGE = 6          # iters per block that stage pt (PV deferred, win banks free)


def _body(tc, xq_d, xk_d, xv_d, wqt_d, wkt_d, wvt_d, bq_d, bk_d, bv_d, wot_d,
          out_d, sums_d, rsums_d):
    nc = tc.nc

    from contextlib import ExitStack
    with ExitStack() as ctx:
        pers = ctx.enter_context(tc.tile_pool(name="pers", bufs=1))
        # per-m-slice tensors are separate tiles so subtile dependency
        # tracking between window jobs and the attention stream stays 2D
        qt0 = pers.tile([P, T], BF16, tag="qt0")
        qt1 = pers.tile([P, T], BF16, tag="qt1")
        kt0 = pers.tile([P, T], BF16, tag="kt0")
        kt1 = pers.tile([P, T], BF16, tag="kt1")
        v_sb = pers.tile([P, TC, NHL, 68], BF16, tag="v")
        attn0 = pers.tile([P, T], BF16, tag="attn0")
        attn1 = pers.tile([P, T], BF16, tag="attn1")
        wot_sb = pers.tile([P, 2, D], BF16, tag="wot")
        bq_sb = pers.tile([P, 2], F32, tag="bq")
        bk_sb = pers.tile([P, 2], F32, tag="bk")
        bv_sb = pers.tile([P, NHL, HEAD_DIM], F32, tag="bv")
        zero_sb = pers.tile([P, 1], F32, tag="zero")
        scr_sb = pers.tile([P, 1], F32, tag="scr")
        wq_sb = pers.tile([P, DC, S], BF16, tag="wq")
        wk_sb = pers.tile([P, DC, S], BF16, tag="wk")
        wv_sb = pers.tile([P, DC, S], BF16, tag="wv")
        xq_sb = pers.tile([P, DC, T], BF16, tag="xq")
        xk_sb = pers.tile([P, DC, T], BF16, tag="xk")
        xv_sb = pers.tile([P, DC, T], BF16, tag="xv")
        qts = (qt0, qt1)
        kts = (kt0, kt1)
        attns = (attn0, attn1)

        # ---------------- DMA issue (priority order) ----------------
        nc.sync.dma_start(bk_sb[:], bk_d[:])
        nc.sync.dma_start(bq_sb[:], bq_d[:])
        nc.sync.dma_start(bv_sb[:], bv_d[:])
        nc.vector.memset(zero_sb[:], 0.0)
        # Preload the exp table set (~2.7us) so the first real exp doesn't
        # stall the attention pipeline.
        nc.scalar.activation(scr_sb[:], zero_sb[:], AT.Exp,
                             bias=zero_sb[:, 0:1], scale=1.0)
        # ones column for the P~V sum trick (col 64 of every (tchunk, head))
        nc.vector.memset(v_sb[:, :, :, 64:65], 1.0)

        for c in range(DC):
            nc.sync.dma_start(wk_sb[:, c], wkt_d[:, c])
        for c in range(DC):
            nc.sync.dma_start(wq_sb[:, c], wqt_d[:, c])

        def dma_x(sb, dr, g):
            nc.sync.dma_start(sb[:, :, g * TG:(g + 1) * TG],
                              dr[:, :, g * TG:(g + 1) * TG])

        dma_x(xk_sb, xk_d, 0)
        dma_x(xq_sb, xq_d, 0)
        dma_x(xq_sb, xq_d, 1)
        for c in range(DC):
            nc.sync.dma_start(wv_sb[:, c], wvt_d[:, c])
        dma_x(xv_sb, xv_d, 0)
        dma_x(xv_sb, xv_d, 1)
        dma_x(xk_sb, xk_d, 1)
        dma_x(xk_sb, xk_d, 2)
        dma_x(xk_sb, xk_d, 3)
        dma_x(xv_sb, xv_d, 2)
        dma_x(xv_sb, xv_d, 3)
        dma_x(xq_sb, xq_d, 2)
        dma_x(xq_sb, xq_d, 3)
        nc.sync.dma_start(wot_sb[:], wot_d[:])

        # ---------------- pools ----------------
        stp = ctx.enter_context(tc.tile_pool(name="stp", bufs=2, space="PSUM"))
        winp = ctx.enter_context(tc.tile_pool(name="winp", bufs=4,
                                              space="PSUM"))
        ptp = ctx.enter_context(tc.tile_pool(name="ptp", bufs=14))
        asbp = ctx.enter_context(tc.tile_pool(name="asb", bufs=4))
        brdp = ctx.enter_context(tc.tile_pool(name="brd", bufs=2))
        rcpp = ctx.enter_context(tc.tile_pool(name="rcp", bufs=4))
        osbp = ctx.enter_context(tc.tile_pool(name="osb", bufs=3))

        # ---------------- window jobs ----------------
        # Each job borrows one PSUM bank as its accumulator — normally a
        # "win" slot (free during each block's stage phase), but jobs that
        # must be force-emitted mid-block (safety net) use an "st" slot
        # instead: the win slots may hold live att accumulators whose
        # evacuation hasn't been emitted yet, while st slots' last tenants
        # are always already-emitted exps. Costs are in mm-512 units
        # (~213ns warm) for per-iter metering.
        def kq_job(x_sb, w_sb, b_sb, dst, m, g):
            def emit(pool):
                ps = pool.tile([P, TG], F32, tag="win" if pool is winp
                               else "st", name="pj")
                for d8 in range(DC):
                    nc.tensor.matmul(
                        ps[:],
                        lhsT=w_sb[:, d8, m * P:(m + 1) * P],
                        rhs=x_sb[:, d8, g * TG:(g + 1) * TG],
                        start=(d8 == 0), stop=(d8 == DC - 1))
                nc.vector.tensor_scalar(
                    dst[:, g * TG:(g + 1) * TG], ps[:],
                    b_sb[:, m:m + 1], None, op0=ALU.add)
            return (emit, 9.0)

        def v_job(t16):
            def emit(pool):
                ps = pool.tile([P, S], F32, tag="win" if pool is winp
                               else "st", name="pv")
                for d8 in range(DC):
                    nc.tensor.matmul(
                        ps[:],
                        lhsT=xv_sb[:, d8, t16 * P:(t16 + 1) * P],
                        rhs=wv_sb[:, d8, :],
                        start=(d8 == 0), stop=(d8 == DC - 1))
                nc.vector.tensor_tensor(
                    v_sb[:, t16, :, 0:64],
                    ps[:].rearrange("p (h x) -> p h x", h=NHL),
                    bv_sb[:], op=ALU.add)
            return (emit, 4.5)

        ob_tiles = {}

        def op_job(m, n):
            def emit(pool):
                ps = pool.tile([P, TG], F32, tag="win" if pool is winp
                               else "st", name="po")
                for sc in range(2):
                    nc.tensor.matmul(
                        ps[:],
                        lhsT=attns[sc][:, m * P:(m + 1) * P],
                        rhs=wot_sb[:, sc, n * 512:(n + 1) * 512],
                        start=(sc == 0), stop=(sc == 1))
                if n == 0:
                    ob_tiles[m] = osbp.tile([P, D], BF16, tag="ob", name="ob")
                ob = ob_tiles[m]
                nc.vector.tensor_copy(ob[:, n * 512:(n + 1) * 512], ps[:])
                if n == 1:
                    nc.sync.dma_start(out_d[m], ob[:])
                    del ob_tiles[m]
            return (emit, 2.5)

        jobs = []          # [key, emit(pool), cost]

        def push(key, je):
            jobs.append([key, je[0], je[1]])

        def force(key):
            # Emergency in-order emission of a prerequisite job (uses an st
            # slot; see above). No-op if the job already ran.
            for idx, (k, e, _) in enumerate(jobs):
                if k == key:
                    jobs.pop(idx)
                    e(stp)
                    return

        def run_jobs(budget):
            while jobs and budget >= jobs[0][2]:
                _, e, cost = jobs.pop(0)
                e(winp)
                budget -= cost

        # ---------------- prefix ----------------
        # Projections the first block can't start without, emitted directly
        # (win banks are all free here): kt[0] g0/g1 cover QK chunks 0..7,
        # qt[0] g0/g1 is q-half 0, V t0..t8 feeds the first PV catch-ups.
        for emit, _ in [kq_job(xk_sb, wk_sb, bk_sb, kt0, 0, 0),
                        kq_job(xq_sb, wq_sb, bq_sb, qt0, 0, 0),
                        kq_job(xq_sb, wq_sb, bq_sb, qt0, 0, 1),
                        kq_job(xk_sb, wk_sb, bk_sb, kt0, 0, 1),
                        v_job(0), v_job(1), v_job(2), v_job(3),
                        v_job(4), v_job(5), v_job(6), v_job(7), v_job(8)]:
            emit(winp)

        # Remaining projection work, in consumption order (blocks run
        # mh-major: (0,0),(0,1),(1,0),(1,1)). kt[m] group g is first read by
        # QK chunk 4g of an mh=m block; V t by the PV catch-up of block 1
        # (~iter STAGE + t/2); qt[m] groups (2jh, 2jh+1) by block (m, jh).
        # Out-projection jobs are appended once a q-half is normalized.
        push(("v", 9), v_job(9))
        push(("v", 10), v_job(10))
        push(("v", 11), v_job(11))
        push(("v", 12), v_job(12))
        push(("v", 13), v_job(13))
        push(("v", 14), v_job(14))
        push(("v", 15), v_job(15))
        push(("kt", 0, 2), kq_job(xk_sb, wk_sb, bk_sb, kt0, 0, 2))
        push(("kt", 0, 3), kq_job(xk_sb, wk_sb, bk_sb, kt0, 0, 3))
        push(("qt", 0, 2), kq_job(xq_sb, wq_sb, bq_sb, qt0, 0, 2))
        push(("qt", 0, 3), kq_job(xq_sb, wq_sb, bq_sb, qt0, 0, 3))
        push(("kt", 1, 0), kq_job(xk_sb, wk_sb, bk_sb, kt1, 1, 0))
        push(("kt", 1, 1), kq_job(xk_sb, wk_sb, bk_sb, kt1, 1, 1))
        push(("kt", 1, 2), kq_job(xk_sb, wk_sb, bk_sb, kt1, 1, 2))
        push(("kt", 1, 3), kq_job(xk_sb, wk_sb, bk_sb, kt1, 1, 3))
        push(("qt", 1, 0), kq_job(xq_sb, wq_sb, bq_sb, qt1, 1, 0))
        push(("qt", 1, 1), kq_job(xq_sb, wq_sb, bq_sb, qt1, 1, 1))
        push(("qt", 1, 2), kq_job(xq_sb, wq_sb, bq_sb, qt1, 1, 2))
        push(("qt", 1, 3), kq_job(xq_sb, wq_sb, bq_sb, qt1, 1, 3))

        # ---------------- fused attention pipeline ----------------
        def emit_pv(atts_, mh, c, first, pts_):
            pt_A, pt_B = pts_
            for n in range(2):
                ns = slice(n * 512, (n + 1) * 512)
                nc.tensor.matmul(
                    atts_[0][n][0:65, :], lhsT=v_sb[:, c, 2 * mh, 0:65],
                    rhs=pt_A[:, ns], start=first, stop=(c == TC - 1))
                nc.tensor.matmul(
                    atts_[1][n][0:65, :], lhsT=v_sb[:, c, 2 * mh + 1, 0:65],
                    rhs=pt_B[:, ns], start=first, stop=(c == TC - 1))

        def emit_block_tail(ent):
            atts_, mh, jh = ent
            q0 = jh * QH
            # Evacuate all four PSUM accumulators first (frees the win banks
            # for the next block's window before the slower norm chains run).
            attsbs = {}
            for hb in (0, 1):
                for n in (0, 1):
                    attsb = asbp.tile([65, 512], F32, tag="attsb",
                                      name=f"attsb{hb}{n}")
                    nc.vector.tensor_copy(attsb[:], atts_[hb][n][0:65, :])
                    attsbs[(hb, n)] = attsb
            for hb in (0, 1):
                h = 2 * mh + hb
                ph = hb * 64
                # Softmax denominators: reciprocal in partition-major shape,
                # then a partition broadcast — both via DRAM bounces.
                for n in (0, 1):
                    nc.sync.dma_start(
                        sums_d[h:h + 1, q0 + n * 512:q0 + (n + 1) * 512],
                        attsbs[(hb, n)][64:65, :])
                sp = rcpp.tile([P, QH // P], F32, tag="sp")
                nc.sync.dma_start(
                    sp[:], sums_d[h, q0:q0 + QH].rearrange("(p f) -> p f", p=P))
                rp = rcpp.tile([P, QH // P], F32, tag="rp")
                nc.vector.reciprocal(rp[:], sp[:])
                nc.sync.dma_start(
                    rsums_d[h, q0:q0 + QH].rearrange("(p f) -> p f", p=P),
                    rp[:])
                rc = brdp.tile([64, QH], F32, tag="rc")
                nc.sync.dma_start(
                    rc[:], rsums_d[h:h + 1, q0:q0 + QH].broadcast_to((64, QH)))
                for n in (0, 1):
                    nc.vector.tensor_tensor(
                        attns[mh][ph:ph + 64,
                                  q0 + n * 512:q0 + (n + 1) * 512],
                        attsbs[(hb, n)][0:64, :],
                        rc[:, n * 512:(n + 1) * 512], op=ALU.mult)

        blocks = [(0, 0), (0, 1), (1, 0), (1, 1)]
        prev_tail = None       # (atts, mh, jh) awaiting evac + norm
        prev_last_pv = None    # (atts, mh, pts15) final PV still pending
        for bi, (mh, jh) in enumerate(blocks):
            q0 = jh * QH
            atts = None
            pts = {}
            pv_done = 0
            # safety net: this block's qt half must exist before its QKs
            force(("qt", mh, 2 * jh))
            force(("qt", mh, 2 * jh + 1))
            for i in range(TC):
                force(("kt", mh, i // 4))
                st_A = stp.tile([P, QH], F32, tag="st", name="st_A")
                st_B = stp.tile([P, QH], F32, tag="st", name="st_B")
                for n in range(2):
                    ns = slice(n * 512, (n + 1) * 512)
                    qs = slice(q0 + n * 512, q0 + (n + 1) * 512)
                    nc.tensor.matmul(
                        st_A[:, ns], lhsT=kts[mh][0:64, i * P:(i + 1) * P],
                        rhs=qts[mh][0:64, qs], start=True, stop=True)
                    nc.tensor.matmul(
                        st_B[:, ns], lhsT=kts[mh][64:128, i * P:(i + 1) * P],
                        rhs=qts[mh][64:128, qs], start=True, stop=True)
                pt_A = ptp.tile([P, QH], BF16, tag="pt", name="pt_A")
                nc.scalar.activation(pt_A[:], st_A[:], AT.Exp,
                                     bias=zero_sb[:, 0:1], scale=float(SCALE))
                pt_B = ptp.tile([P, QH], BF16, tag="pt", name="pt_B")
                nc.scalar.activation(pt_B[:], st_B[:], AT.Exp,
                                     bias=zero_sb[:, 0:1], scale=float(SCALE))
                pts[i] = (pt_A, pt_B)

                # previous block's software-pipelined tail
                if i == 0 and prev_last_pv is not None:
                    p_atts, p_mh, p_pts = prev_last_pv
                    force(("v", TC - 1))
                    emit_pv(p_atts, p_mh, TC - 1, False, p_pts)
                    prev_last_pv = None
                if i == 1 and prev_tail is not None:
                    emit_block_tail(prev_tail)
                    prev_tail = None
                    if bi == 3:
                        # q-half 0 fully normalized -> its out-projection
                        jobs.extend([[None] + list(op_job(m, n))
                                     for m in range(8) for n in (0, 1)])

                if i < STAGE:
                    # stage mode: win banks belong to window jobs. No jobs
                    # at a boundary iter 0 — the previous block's att banks
                    # haven't had their evacuation emitted yet.
                    if bi == 0:
                        run_jobs(9.0)
                    else:
                        run_jobs(0.0 if i == 0 else (6.0 if i == 1 else 9.0))
                else:
                    if atts is None:
                        atts = [[winp.tile([P, 512], F32, tag="win",
                                           name=f"att{hb}{n}")
                                 for n in (0, 1)] for hb in (0, 1)]
                    # catch-up: consume staged pts at 2 chunks/iter, holding
                    # the final chunk back for the next block's iter 0
                    tgt = min(2 * (i - STAGE + 1), i + 1, TC - 1)
                    while pv_done < tgt:
                        force(("v", pv_done))
                        emit_pv(atts, mh, pv_done, pv_done == 0,
                                pts[pv_done])
                        pts.pop(pv_done, None)
                        pv_done += 1
                    run_jobs(0.0 if pv_done < TC - 1 else 9.5)
            prev_last_pv = (atts, mh, pts[TC - 1])
            prev_tail = (atts, mh, jh)

        # ---------------- drain tail ----------------
        p_atts, p_mh, p_pts = prev_last_pv
        emit_pv(p_atts, p_mh, TC - 1, False, p_pts)
        emit_block_tail(prev_tail)
        jobs.extend([[None] + list(op_job(m, n))
                     for m in range(8, 16) for n in (0, 1)])
        run_jobs(1e9)


def _shard_inputs(query, key, value, wq, bq, wk, bk, wv, bv, wo):
    """Build the 8 per-core input maps (all host-side numpy)."""
    bf16 = ml_dtypes.bfloat16
    in_maps = []

    def fold_dmajor(a_t, inner):
        # (D, inner) -> [P, DC, inner]
        return np.ascontiguousarray(
            a_t.reshape(DC, P, inner).transpose(1, 0, 2))

    xs = {}
    for b in range(B):
        for name, x in (("xq", query), ("xk", key), ("xv", value)):
            xt = np.ascontiguousarray(x[b].T).astype(bf16)  # (D, T)
            xs[(name, b)] = fold_dmajor(xt, T)

    for c in range(N_CORES):
        b, g = divmod(c, NHL)
        gs = g * S
        wq_g = wq[gs:gs + S]          # (S, D)
        wk_g = wk[gs:gs + S]
        wv_g = wv[gs:gs + S]
        wo_g = wo[:, gs:gs + S]       # (D, S)
        m = {
            "xq": xs[("xq", b)],
            "xk": xs[("xk", b)],
            "xv": xs[("xv", b)],
            "wqt": fold_dmajor(np.ascontiguousarray(wq_g.T).astype(bf16), S),
            "wkt": fold_dmajor(np.ascontiguousarray(wk_g.T).astype(bf16), S),
            "wvt": fold_dmajor(np.ascontiguousarray(wv_g.T).astype(bf16), S),
            "bq": np.ascontiguousarray(
                bq[gs:gs + S].reshape(2, P).T).astype(np.float32),
            "bk": np.ascontiguousarray(
                bk[gs:gs + S].reshape(2, P).T).astype(np.float32),
            "bv": np.ascontiguousarray(np.broadcast_to(
                bv[gs:gs + S].reshape(NHL, HEAD_DIM), (P, NHL, HEAD_DIM))
            ).astype(np.float32),
            "wot": np.ascontiguousarray(
                wo_g.T.reshape(2, P, D).transpose(1, 0, 2)).astype(bf16),
        }
        in_maps.append(m)
    return in_maps


def _reference_numpy(query, key, value, mask, wq, bq, wk, bk, wv, bv, wo, bo):
    """Pure-numpy fallback for non-trivial masks (never hit for spec inputs)."""
    def lin(x, w, b):
        return np.einsum("btd,od->bto", x, w) + b
    Bq, Tq, _ = query.shape
    Q = lin(query, wq, bq).reshape(Bq, Tq, N_HEADS, HEAD_DIM).transpose(0, 2, 1, 3)
    K = lin(key, wk, bk).reshape(Bq, Tq, N_HEADS, HEAD_DIM).transpose(0, 2, 1, 3)
    V = lin(value, wv, bv).reshape(Bq, Tq, N_HEADS, HEAD_DIM).transpose(0, 2, 1, 3)
    scores = np.einsum("bhqd,bhkd->bhqk", Q, K) * SCALE
    scores = np.where(mask[:, None, :, :] == 0, -np.inf, scores)
    scores = scores - scores.max(axis=-1, keepdims=True)
    e = np.exp(scores)
    probs = e / e.sum(axis=-1, keepdims=True)
    att = np.einsum("bhqk,bhkd->bhqd", probs, V)
    att = att.transpose(0, 2, 1, 3).reshape(Bq, Tq, N_HEADS * HEAD_DIM)
    return (np.einsum("btd,od->bto", att, wo) + bo).astype(np.float32)


def _enable_local_tracing():
    """Make bass_utils' axon NTFF-trace path work in this container:
    register the ctypes profile hook under the missing antenv.axon_hooks
    name and keep artifacts local instead of uploading."""
    import sys
    import types
    try:
        import antenv.axon_hooks  # noqa: F401
    except Exception:
        try:
            from trn_agent_boot.trn_boot import _ntff_profile_via_ctypes
            hook = _ntff_profile_via_ctypes("/opt/axon/libaxon_pjrt.so")
            if hook is None:
                return False
            holder = {"hook": hook}
            m2 = types.ModuleType("antenv.axon_hooks")
            m2.get_axon_ntff_profile_hook = lambda: holder["hook"]
            m2.set_axon_ntff_profile_hook = lambda h: holder.update(hook=h)
            if "antenv" not in sys.modules:
                m1 = types.ModuleType("antenv")
                m1.axon_hooks = m2
                sys.modules["antenv"] = m1
            else:
                sys.modules["antenv"].axon_hooks = m2
            sys.modules["antenv.axon_hooks"] = m2
        except Exception:
            return False
    bass_utils.upload_artifacts = lambda tmpdir: tmpdir
    return True


def kernel(query, key, value, mask, wq, bq, wk, bk, wv, bv, wo, bo):
    query = np.asarray(query, np.float32)
    key = np.asarray(key, np.float32)
    value = np.asarray(value, np.float32)
    wq_, bq_ = np.asarray(wq, np.float32), np.asarray(bq, np.float32)
    wk_, bk_ = np.asarray(wk, np.float32), np.asarray(bk, np.float32)
    wv_, bv_ = np.asarray(wv, np.float32), np.asarray(bv, np.float32)
    wo_, bo_ = np.asarray(wo, np.float32), np.asarray(bo, np.float32)
    mask_np = np.asarray(mask)

    if not np.all(mask_np != 0):
        # Spec inputs always have an all-ones mask; keep a correct fallback.
        return _reference_numpy(query, key, value, mask_np, wq_, bq_,
                                wk_, bk_, wv_, bv_, wo_, bo_)

    # Experimental only: walrus's LDW opt rejects some of our weight loads.
    if os.environ.get("KERNEL_LDW_OPT", "0") == "1":
        _patch_walrus_flags()

    if "prog" not in _CACHE:
        _CACHE["prog"] = _build_program()
    nc = _CACHE["prog"]

    in_maps = _shard_inputs(query, key, value, wq_, bq_, wk_, bk_, wv_, bv_, wo_)

    trace = os.environ.get("KERNEL_TRACE", "0") == "1"
    kw = {}
    if trace:
        trace = _enable_local_tracing()
        if trace:
            tdir = os.environ.get("KERNEL_TRACE_DIR")
            if tdir:
                os.makedirs(tdir, exist_ok=True)
                kw["tmpdir"] = tdir
    try:
        res = bass_utils.run_bass_kernel_spmd(
            nc, in_maps, core_ids=list(range(N_CORES)), trace=trace, **kw)
    except Exception:
        if not trace:
            raise
        import traceback
        traceback.print_exc()
        res = bass_utils.run_bass_kernel_spmd(
            nc, in_maps, core_ids=list(range(N_CORES)), trace=False)

    LAST_STATS.clear()
    LAST_STATS["exec_time_ns"] = res.exec_time_ns
    LAST_STATS["profile_json"] = res.profile_json
    if res.instructions_and_trace is not None:
        LAST_STATS["trace_url"] = res.instructions_and_trace[1]

    out = np.empty((B, T, D), np.float32)
    for b in range(B):
        acc = np.zeros((T, D), np.float32)
        for g in range(NHL):
            acc += res.results[b * NHL + g]["out_part"].reshape(T, D).astype(
                np.float32)
        out[b] = acc + bo_
    return out

